# revision 1
# baseline (speedup 1.0000x reference)
"""Trainium2 Bass kernel: distributed GIN graph encoder on 8 NeuronCores.

Self-contained: host-side graph partitioning + index/one-hot table construction,
Bass/Tile graph build, SPMD execution via run_bass_kernel_spmd, result gather.
"""

import numpy as np
import ml_dtypes

BF = ml_dtypes.bfloat16

DEF_CFG = dict(
    W=8, H=128, L=4,
    n_sub=30000, S_sub=30080,     # per-core real/padded sub rows
    n_glob=3750, S_glob=3840,
    VA=128, G=300, TEMP=0.5, BN_EPS=1e-5,
    SEC_CAP=32768,                # stage-2 idx space cap per section
    G1CALL=6144,                  # stage-1 gather call size
    GRP=16,                       # stage-2 chunks per gather call
)


def _pack16(arr):
    """idx array (n,) int -> [128, n/16] int16 tile content (pos i -> [i%16, i//16])."""
    a = np.asarray(arr, np.int16)
    assert len(a) % 16 == 0
    t = a.reshape(-1, 16).T
    return np.tile(t, (8, 1))


def _pad128(n):
    return (n + 127) // 128 * 128


def build_plan(cfg, x, edge_index, sub_node_map, sub_edge_index, root_idx,
               target_batch, batch):
    """All index tables / budgets, shared-static structure + per-core data."""
    W, H = cfg["W"], cfg["H"]
    n_sub, S_sub = cfg["n_sub"], cfg["S_sub"]
    n_glob, S_glob = cfg["n_glob"], cfg["S_glob"]
    NS = n_sub * W
    N = n_glob * W
    plan = {"cfg": cfg}

    # ---------- sub phase edge plan ----------
    src, dst = np.asarray(sub_edge_index[0]), np.asarray(sub_edge_index[1])
    owner = dst // n_sub
    dst_local = dst % n_sub
    src_row = (src // n_sub) * S_sub + (src % n_sub)   # row in padded replica
    n_tiles_sub = S_sub // 128

    # per (core, tile): edge lists; global chunk budget per tile
    per_core = []
    for c in range(W):
        m = owner == c
        sc, dl = src_row[m], dst_local[m]
        order = np.argsort(dl, kind="stable")
        per_core.append((sc[order], dl[order]))
    Ct = np.zeros(n_tiles_sub, np.int64)   # chunks per tile (global max)
    tile_counts = []
    for c in range(W):
        _, dl = per_core[c]
        cnt = np.bincount(dl // 128, minlength=n_tiles_sub)
        tile_counts.append(cnt)
        Ct = np.maximum(Ct, (cnt + 127) // 128)
    Ct = np.maximum(Ct, 1)

    # sections: runs of tiles with sum(Ct)*128 <= SEC_CAP
    sections = []  # list of (t0, t1, n_slots)
    t0 = 0
    while t0 < n_tiles_sub:
        t1, acc = t0, 0
        while t1 < n_tiles_sub and (acc + Ct[t1]) * 128 <= cfg["SEC_CAP"]:
            acc += Ct[t1]
            t1 += 1
        sections.append((t0, t1, int(acc) * 128))
        t0 = t1
    plan["sub_sections"] = sections
    plan["sub_Ct"] = Ct
    tile_slot_off = np.zeros(n_tiles_sub + 1, np.int64)
    tile_slot_off[1:] = np.cumsum(Ct * 128)

    # stage-1: per section, per bucket: budget (max over cores, pad128)
    # stage-2: chunk slots per tile; per-core content
    g1_budget = np.zeros((len(sections), W), np.int64)
    core_sec = []   # per core: per section: (src_rows_sorted_by_bucket, slot_pos_of_each, dst_local)
    for c in range(W):
        sc, dl = per_core[c]
        tl = dl // 128
        secs = []
        for si, (ta, tb, _) in enumerate(sections):
            m = (tl >= ta) & (tl < tb)
            s_sc, s_dl = sc[m], dl[m]
            b = s_sc // S_sub
            for bb in range(W):
                g1_budget[si, bb] = max(g1_budget[si, bb], _pad128(int((b == bb).sum())))
            secs.append((s_sc, s_dl))
        core_sec.append(secs)
    plan["g1_budget"] = g1_budget

    # build per-core packed idx streams + S matrices
    TOTCH = int(Ct.sum())
    plan["sub_TOTCH"] = TOTCH
    g1_off = np.zeros((len(sections), W + 1), np.int64)  # row offsets in section buffer
    for si in range(len(sections)):
        g1_off[si, 1:] = np.cumsum(g1_budget[si])
    plan["g1_off"] = g1_off
    sec_rows = g1_off[:, -1]                      # stage-1 rows per section
    plan["sub_sec_rows"] = sec_rows
    sec_base = np.zeros(len(sections) + 1, np.int64)
    sec_base[1:] = np.cumsum(sec_rows)
    plan["sub_sec_base"] = sec_base
    assert sec_rows.max() <= 32768

    g1_idx_cores, g2_idx_cores, S_cores = [], [], []
    for c in range(W):
        g1_stream = []
        g2 = np.zeros(TOTCH * 128, np.int64)
        S = np.zeros((TOTCH * 128, 128), BF)
        for si, (ta, tb, _) in enumerate(sections):
            s_sc, s_dl = core_sec[c][si]
            b = s_sc // S_sub
            # stage-1: bucket-major layout in section buffer
            pos_in_sec = np.zeros(len(s_sc), np.int64)
            for bb in range(W):
                mb = b == bb
                nb = int(mb.sum())
                pad_n = g1_budget[si, bb]
                run = np.zeros(pad_n, np.int64)
                run[:nb] = s_sc[mb] % S_sub
                g1_stream.append(run)
                pos_in_sec[mb] = g1_off[si, bb] + np.arange(nb)
            # stage-2: per tile, chunk slots
            for t in range(ta, tb):
                mt = (s_dl // 128) == t
                k = int(mt.sum())
                slots = tile_slot_off[t] + np.arange(k)
                g2[slots] = pos_in_sec[mt]
                S[slots, s_dl[mt] % 128] = BF(1.0)
        g1_idx_cores.append(_pack16(np.concatenate(g1_stream)))
        g2_idx_cores.append(_pack16(g2))
        S_cores.append(S)
    g1_sec_cols = np.zeros(len(sections) + 1, np.int64)
    for si in range(len(sections)):
        g1_sec_cols[si + 1] = g1_sec_cols[si] + int(g1_budget[si].sum()) // 16
    plan["g1_sec_cols"] = g1_sec_cols
    plan["sub_g1_idx"] = g1_idx_cores
    plan["sub_g2_idx"] = g2_idx_cores
    plan["sub_S"] = [S.reshape(TOTCH, 128, 128).transpose(1, 0, 2).reshape(128, TOTCH * 128)
                     for S in S_cores]
    plan["sub_G1TOT"] = g1_idx_cores[0].shape[1] * 16

    # ---------- glob phase edge plan (single stage) ----------
    gsrc, gdst = np.asarray(edge_index[0]), np.asarray(edge_index[1])
    gowner = gdst // n_glob
    gdst_local = gdst % n_glob
    gsrc_row = (gsrc // n_glob) * S_glob + (gsrc % n_glob)
    n_tiles_glob = S_glob // 128
    per_core_g = []
    CtG = np.zeros(n_tiles_glob, np.int64)
    for c in range(W):
        m = gowner == c
        sc, dl = gsrc_row[m], gdst_local[m]
        order = np.argsort(dl, kind="stable")
        sc, dl = sc[order], dl[order]
        per_core_g.append((sc, dl))
        cnt = np.bincount(dl // 128, minlength=n_tiles_glob)
        CtG = np.maximum(CtG, (cnt + 127) // 128)
    CtG = np.maximum(CtG, 1)
    plan["glob_Ct"] = CtG
    TOTCHG = int(CtG.sum())
    plan["glob_TOTCH"] = TOTCHG
    tile_slot_off_g = np.zeros(n_tiles_glob + 1, np.int64)
    tile_slot_off_g[1:] = np.cumsum(CtG * 128)
    gg_idx_cores, Sg_cores = [], []
    for c in range(W):
        sc, dl = per_core_g[c]
        g2 = np.zeros(TOTCHG * 128, np.int64)
        S = np.zeros((TOTCHG * 128, 128), BF)
        tl = dl // 128
        for t in range(n_tiles_glob):
            mt = tl == t
            k = int(mt.sum())
            slots = tile_slot_off_g[t] + np.arange(k)
            g2[slots] = sc[mt]
            S[slots, dl[mt] % 128] = BF(1.0)
        gg_idx_cores.append(_pack16(g2))
        Sg_cores.append(S)
    plan["glob_g2_idx"] = gg_idx_cores
    plan["glob_S"] = [S.reshape(TOTCHG, 128, 128).transpose(1, 0, 2).reshape(128, TOTCHG * 128)
                      for S in Sg_cores]

    # ---------- atom encode ----------
    aid = np.asarray(x)[np.asarray(sub_node_map)]      # [NS] atom id per sub node
    plan["aid"] = []
    for c in range(W):
        a = np.zeros(S_sub, np.int64)
        a[:n_sub] = aid[c * n_sub:(c + 1) * n_sub]
        plan["aid"].append(_pack16(a))

    # ---------- phase boundary (roots) ----------
    tb_arr = np.asarray(target_batch)
    ri = np.asarray(root_idx)
    order = np.argsort(tb_arr, kind="stable")
    assert (np.bincount(tb_arr, minlength=N) == 2).all(), "need exactly 2 roots/node"
    r_sorted = ri[order].reshape(N, 2)      # root rows (in hsub global) per node
    lp_order = order.reshape(N, 2)          # positions into log_probs
    plan["r0"], plan["r4"], plan["lp_sel"] = [], [], []
    for c in range(W):
        r = r_sorted[c * n_glob:(c + 1) * n_glob]
        lo = c * n_sub
        assert ((r >= lo) & (r < lo + n_sub)).all(), "roots must be core-local"
        r0 = np.zeros(S_glob, np.int64)
        r4 = np.zeros(S_glob, np.int64)
        r0[:n_glob] = r[:, 0] - lo
        r4[:n_glob] = r[:, 1] - lo
        plan["r0"].append(_pack16(r0))
        plan["r4"].append(_pack16(r4))
        plan["lp_sel"].append(lp_order[c * n_glob:(c + 1) * n_glob])  # [n_glob,2]

    # ---------- readout ----------
    b_arr = np.asarray(batch)
    plan["Sg"] = []
    for c in range(W):
        Srd = np.zeros((S_glob, cfg["G"]), BF)
        ids = b_arr[c * n_glob:(c + 1) * n_glob]
        Srd[np.arange(n_glob), ids] = BF(1.0)
        nt = S_glob // 128
        plan["Sg"].append(Srd.reshape(nt, 128, cfg["G"]).transpose(1, 0, 2).reshape(128, nt * cfg["G"]))
    return plan




def _install_queue_aware_lanes():
    """Make Tile's DMASW lane assignment queue-aware: lane = queue*2 + rr.
    Needed because we spread dma_gather over 4 SWDGE queues; the stock
    assigner round-robins 8 lanes queue-blind, which trips the per-queue
    sem lock in ucode/sim."""
    import concourse.tile_sem_assignment as tsa
    if getattr(tsa, "_qaware_installed", False):
        return
    orig = tsa.TileClockTick._assign_tick
    import concourse.mybir as mb

    def patched(self, inst):
        qn = getattr(inst, "queue_num", None)
        if (qn is not None and inst.engine == mb.EngineType.Pool
                and isinstance(inst, tsa.DMAInst)
                and self.swdge_sem_count == 8):
            rr_map = getattr(self, "_q_rr", None)
            if rr_map is None:
                rr_map = self._q_rr = {}
            sub = rr_map.get(qn, 0)
            rr_map[qn] = (sub + 1) % 2
            lane = qn * 2 + sub
            save = self.next_sw_dma_idx
            self.next_sw_dma_idx = lane
            try:
                return orig(self, inst)
            finally:
                self.next_sw_dma_idx = save
        return orig(self, inst)

    tsa.TileClockTick._assign_tick = patched
    tsa._qaware_installed = True


def build_graph(plan):
    from concourse import bass, mybir, bacc
    import concourse.tile as tile

    cfg = plan["cfg"]
    W, H, L = cfg["W"], cfg["H"], cfg["L"]
    n_sub, S_sub = cfg["n_sub"], cfg["S_sub"]
    n_glob, S_glob = cfg["n_glob"], cfg["S_glob"]
    G = cfg["G"]
    BF16 = mybir.dt.bfloat16
    F32 = mybir.dt.float32
    I16 = mybir.dt.int16
    AF = mybir.ActivationFunctionType
    OP = mybir.AluOpType
    GRP = cfg["GRP"]

    _install_queue_aware_lanes()
    nc = bacc.Bacc("TRN2", target_bir_lowering=False, debug=False, num_devices=W,
                   num_swdge_queues=4)

    # ---- inputs ----
    def inp(name, shape, dt):
        return nc.dram_tensor(name, shape, dt, kind="ExternalInput")

    G1TOT = plan["sub_G1TOT"]
    TOTCH, TOTCHG = plan["sub_TOTCH"], plan["glob_TOTCH"]
    t_g1 = inp("g1idx", [128, G1TOT // 16], I16)
    t_g2 = inp("g2idx", [128, TOTCH * 8], I16)
    t_gg = inp("ggidx", [128, TOTCHG * 8], I16)
    t_aid = inp("aid", [128, S_sub // 16], I16)
    t_r0 = inp("r0idx", [128, S_glob // 16], I16)
    t_r4 = inp("r4idx", [128, S_glob // 16], I16)
    t_Ssub = inp("Ssub", [128, TOTCH * 128], BF16)
    t_Sglob = inp("Sglob", [128, TOTCHG * 128], BF16)
    t_Srd = inp("Srd", [128, (S_glob // 128) * G], BF16)
    t_idn_bf = inp("idnbf", [128, 128], BF16)
    t_idn_f = inp("idnf", [128, 128], F32)
    t_atom = inp("atom", [cfg["VA"], H], BF16)
    t_W1s = inp("W1s", [L, H, H], BF16)
    t_W2s = inp("W2s", [L, H, H], BF16)
    t_W1g = inp("W1g", [L, H, H], BF16)
    t_W2g = inp("W2g", [L, H, H], BF16)
    t_vecs = inp("vecs", [128, 10 * L], F32)   # b1,b2,gam,bet,eps per layer x(sub,glob)
    t_lp = inp("lp", [S_glob, 2], F32)
    t_out = nc.dram_tensor("out", [G, H], F32, kind="ExternalOutput")

    # ---- internal DRAM ----
    rep_sub = nc.dram_tensor("rep_sub", [W * S_sub, H], BF16, addr_space="Shared")
    rep_glob = nc.dram_tensor("rep_glob", [W * S_glob, H], BF16, addr_space="Shared")
    hown_sub = nc.dram_tensor("hown_sub", [S_sub, H], BF16)
    hown_glob = nc.dram_tensor("hown_glob", [S_glob, H], BF16)
    sec_rows = plan["sub_sec_rows"]
    sec_base = plan["sub_sec_base"]
    ebuf = nc.dram_tensor("ebuf", [int(sec_base[-1]), H], BF16)
    ar_in = nc.dram_tensor("ar_in", [128, 2], F32)
    ar_out = nc.dram_tensor("ar_out", [128, 2], F32, addr_space="Shared")
    rd_in = nc.dram_tensor("rd_in", [128, G], F32)
    rd_out = nc.dram_tensor("rd_out", [128, G], F32, addr_space="Shared")

    RG = [list(range(W))]
    _qrr = [0]

    def nextq():
        # must mirror tile_sem_assignment's DMASW lane rotation (8 lanes,
        # advanced per Pool-engine DMA inst in trace order): lane i -> queue i//2
        q = (_qrr[0] % 8) // 2
        _qrr[0] += 1
        return q
    sections = plan["sub_sections"]
    Ct, CtG = plan["sub_Ct"], plan["glob_Ct"]
    g1_budget, g1_off = plan["g1_budget"], plan["g1_off"]

    with tile.TileContext(nc) as tc:
        with (
            tc.tile_pool(name="const", bufs=1) as constp,
            tc.tile_pool(name="idx", bufs=1) as idxp,
            tc.tile_pool(name="seg", bufs=2) as segp,
            tc.tile_pool(name="xs", bufs=3) as xsp,
            tc.tile_pool(name="zz", bufs=3) as zzp,
            tc.tile_pool(name="res", bufs=1) as resp,
            tc.tile_pool(name="small", bufs=2) as smp,
            tc.tile_pool(name="stg", bufs=2) as stgp,
            tc.tile_pool(name="psA", bufs=2, space="PSUM") as psA,
            tc.tile_pool(name="psM", bufs=1, space="PSUM") as psM,
            tc.tile_pool(name="psT", bufs=2, space="PSUM") as psT,
        ):
            # ---- constants resident ----
            vecs = constp.tile([128, 10 * L], F32)
            nc.sync.dma_start(vecs[:], t_vecs[:])
            W1s = constp.tile([128, L * H], BF16)
            W2s = constp.tile([128, L * H], BF16)
            W1g = constp.tile([128, L * H], BF16)
            W2g = constp.tile([128, L * H], BF16)
            for l in range(L):
                nc.sync.dma_start(W1s[:, l * H:(l + 1) * H], t_W1s[l])
                nc.sync.dma_start(W2s[:, l * H:(l + 1) * H], t_W2s[l])
                nc.sync.dma_start(W1g[:, l * H:(l + 1) * H], t_W1g[l])
                nc.sync.dma_start(W2g[:, l * H:(l + 1) * H], t_W2g[l])
            idn = constp.tile([128, 128], BF16, tag="idn")
            nc.sync.dma_start(idn[:], t_idn_bf[:])

            # vecs columns: per l: [b1, b2, gamma, beta, eps] sub then glob
            def vcol(phase, l, j):
                return vecs[:, (phase * 5 * L + l * 5 + j):(phase * 5 * L + l * 5 + j) + 1]

            # ---- atom encode -> hown_sub ----
            aid_sb = idxp.tile([128, S_sub // 16], I16, tag="g1", bufs=2)
            nc.sync.dma_start(aid_sb[:], t_aid[:])
            CALL = 8192
            off = 0
            while off < S_sub:
                n = min(CALL, S_sub - off)
                at = segp.tile([128, CALL // 128, H], BF16, tag="seg")
                nc.gpsimd.dma_gather(at[:, :n // 128, :], t_atom[:],
                                     aid_sb[:, off // 16:(off + n) // 16], n, n, H,
                                     single_packet=False, queue_num=nextq())
                nc.sync.dma_start(
                    hown_sub.ap().rearrange("(c p) h -> p c h", p=128)[:, off // 128:(off + n) // 128, :],
                    at[:, :n // 128, :])
                off += n

            def gin_layer(phase, l, last):
                """One GIN layer. phase 0=sub, 1=glob."""
                if phase == 0:
                    Sp, n_real, rep, hown = S_sub, n_sub, rep_sub, hown_sub
                    Wt1, Wt2, t_S, t_gi, Ctp = W1s, W2s, t_Ssub, t_g2, Ct
                    NTOT = n_sub * W
                else:
                    Sp, n_real, rep, hown = S_glob, n_glob, rep_glob, hown_glob
                    Wt1, Wt2, t_S, t_gi, Ctp = W1g, W2g, t_Sglob, t_gg, CtG
                    NTOT = n_glob * W
                n_tiles = Sp // 128

                # AllGather h -> replica
                nc.gpsimd.collective_compute(
                    "AllGather", OP.bypass, RG, [hown.ap().opt()], [rep.ap().opt()])

                # stage-1 (sub only): bucket gathers -> ebuf
                if phase == 0:
                    g1c = plan["g1_sec_cols"]
                    g1w = max(int(g1c[si + 1] - g1c[si]) for si in range(len(sections)))
                    g1w = max(g1w, S_sub // 16)
                    for si in range(len(sections)):
                        g1_sb = idxp.tile([128, g1w], I16, tag="g1", name=f"g1s{si%2}", bufs=2)
                        ncols = int(g1c[si + 1] - g1c[si])
                        nc.sync.dma_start(g1_sb[:, :ncols], t_g1[:, int(g1c[si]):int(g1c[si + 1])])
                        col = 0
                        for b in range(W):
                                nbud = int(g1_budget[si, b])
                                roff = int(g1_off[si, b])
                                done = 0
                                while done < nbud:
                                    n = min(cfg["G1CALL"], nbud - done)
                                    st = segp.tile([128, cfg["G1CALL"] // 128, H], BF16, tag="seg")
                                    nc.gpsimd.dma_gather(
                                        st[:, :n // 128, :],
                                        rep[b * S_sub:(b + 1) * S_sub, :],
                                        g1_sb[:, col:col + n // 16], n, n, H,
                                        single_packet=False, queue_num=nextq())
                                    erow = int(sec_base[si]) + roff + done
                                    nc.sync.dma_start(
                                        ebuf.ap().rearrange("(c p) h -> p c h", p=128)[:, erow // 128:(erow + n) // 128, :],
                                        st[:, :n // 128, :])
                                    col += n // 16
                                    done += n

                gi_sb = idxp.tile([128, (TOTCH if phase == 0 else TOTCHG) * 8], I16, tag="g2")
                nc.sync.dma_start(gi_sb[:], t_gi[:])

                # stage-2 + aggregation matmuls + epilogue + MLP
                z2 = resp.tile([128, S_sub], BF16, tag="z2")     # max-size shared
                stats = smp.tile([128, 64 * 6], F32, tag="stats")
                # chunk schedule
                chunks = []   # (tile, first, last)
                for t in range(n_tiles):
                    for j in range(int(Ctp[t])):
                        chunks.append((t, j == 0, j == int(Ctp[t]) - 1))
                # section row offset for stage-2 idx source (sub): idx local to section
                sec_of_tile = {}
                if phase == 0:
                    for si, (ta, tb, _) in enumerate(sections):
                        for t in range(ta, tb):
                            sec_of_tile[t] = si

                psum_of = {}
                ztile = {}
                groups = [chunks[i:i + GRP] for i in range(0, len(chunks), GRP)]
                ch_base = 0
                n_groups_mlp = (Sp + 511) // 512
                zgrp_tiles = {}
                done_tiles = 0

                def run_mlp(g):
                    zg = zgrp_tiles.pop(g)
                    cols = min(512, Sp - g * 512)
                    pm = psM.tile([128, 512], F32, tag="m1")
                    nc.tensor.matmul(pm[:, :cols], Wt1[:, l * H:(l + 1) * H], zg[:, :cols],
                                     start=True, stop=True)
                    z1 = zzp.tile([128, 512], BF16, tag="z1")
                    nc.scalar.activation(z1[:, :cols], pm[:, :cols], AF.Relu,
                                         bias=vcol(phase, l, 0), scale=1.0)
                    pm2 = psM.tile([128, 512], F32, tag="m2")
                    nc.tensor.matmul(pm2[:, :cols], Wt2[:, l * H:(l + 1) * H], z1[:, :cols],
                                     start=True, stop=True)
                    nc.vector.tensor_scalar(z2[:, g * 512:g * 512 + cols], pm2[:, :cols],
                                            vcol(phase, l, 1), None, op0=OP.add)
                    realc = min(512, max(0, n_real - g * 512))
                    if realc > 0:
                        nc.vector.bn_stats(stats[:, g * 6:(g + 1) * 6],
                                           z2[:, g * 512:g * 512 + realc])

                for gci, grp in enumerate(groups):
                    nch = len(grp)
                    xt = xsp.tile([128, GRP, H], BF16, tag="x")
                    n = nch * 128
                    if phase == 0:
                        si = sec_of_tile[grp[0][0]]
                        # all chunks in one gather must come from one section
                        si_end = sec_of_tile[grp[-1][0]]
                        if si_end != si:
                            # split at section boundary: do per-subrange gathers
                            sub_rs = []
                            i0 = 0
                            for i in range(1, nch):
                                if sec_of_tile[grp[i][0]] != sec_of_tile[grp[i0][0]]:
                                    sub_rs.append((i0, i))
                                    i0 = i
                            sub_rs.append((i0, nch))
                        else:
                            sub_rs = [(0, nch)]
                        for (i0, i1) in sub_rs:
                            nn = (i1 - i0) * 128
                            ssi = sec_of_tile[grp[i0][0]]
                            nc.gpsimd.dma_gather(
                                xt[:, i0:i1, :],
                                ebuf[int(sec_base[ssi]):int(sec_base[ssi] + sec_rows[ssi]), :],
                                gi_sb[:, (ch_base + i0) * 8:(ch_base + i1) * 8],
                                nn, nn, H, single_packet=False, queue_num=nextq())
                    else:
                        nc.gpsimd.dma_gather(
                            xt[:, :nch, :], rep[:, :],
                            gi_sb[:, ch_base * 8:(ch_base + nch) * 8],
                            n, n, H, single_packet=False, queue_num=nextq())
                    st = xsp.tile([128, GRP * 128], BF16, tag="s")
                    nc.sync.dma_start(st[:, :n], t_S[:, ch_base * 128:ch_base * 128 + n])
                    for j, (t, first, lastc) in enumerate(grp):
                        if first:
                            psum_of[t] = psA.tile([128, 128], F32, tag="agg", name=f"agg{t%8}")
                        nc.tensor.matmul(psum_of[t][:], xt[:, j, :],
                                         st[:, j * 128:(j + 1) * 128],
                                         start=first, stop=lastc)
                        if lastc:
                            # epilogue: z^T tile = (1+eps)*h^T + agg^T
                            g = (t * 128) // 512
                            if g not in zgrp_tiles:
                                zgrp_tiles[g] = zzp.tile([128, 512], BF16, tag="zg", name=f"zg{g%4}")
                                hg = zzp.tile([128, 512], BF16, tag="hg")
                                cols = min(512, Sp - g * 512)
                                nc.sync.dma_start(hg[:, :cols],
                                                  hown[g * 512:g * 512 + cols, :],
                                                  transpose=True)
                                zgrp_tiles[(g, "h")] = hg
                            hg = zgrp_tiles[(g, "h")]
                            cc = t * 128 - g * 512
                            pt = psum_of.pop(t)
                            nc.vector.scalar_tensor_tensor(
                                zgrp_tiles[g][:, cc:cc + 128], hg[:, cc:cc + 128],
                                vcol(phase, l, 4), pt[:],
                                op0=OP.mult, op1=OP.add)
                            done_tiles += 1
                            if (t * 128 + 128) % 512 == 0 or t == n_tiles - 1:
                                zgrp_tiles.pop((g, "h"))
                                run_mlp(g)
                    ch_base += nch

                # BN: aggregate + allreduce
                ngr = (n_real + 511) // 512
                mv = smp.tile([128, 2], F32, tag="mv")
                nc.vector.bn_aggr(mv[:], stats[:, :ngr * 6])
                sin = smp.tile([128, 2], F32, tag="sin")
                # sin = [mean, var + mean^2]
                nc.vector.tensor_tensor(sin[:, 1:2], mv[:, 0:1], mv[:, 0:1], op=OP.mult)
                nc.vector.tensor_tensor(sin[:, 1:2], sin[:, 1:2], mv[:, 1:2], op=OP.add)
                nc.vector.tensor_copy(sin[:, 0:1], mv[:, 0:1])
                nc.sync.dma_start(ar_in[:], sin[:])
                nc.gpsimd.collective_compute(
                    "AllReduce", OP.add, RG, [ar_in.ap().opt()], [ar_out.ap().opt()])
                sg = smp.tile([128, 2], F32, tag="sg")
                nc.sync.dma_start(sg[:], ar_out[:])
                mu = smp.tile([128, 4], F32, tag="mu")
                nc.scalar.mul(mu[:, 0:1], sg[:, 0:1], 1.0 / W)          # mu
                nc.scalar.mul(mu[:, 1:2], sg[:, 1:2], 1.0 / W)          # E[z2]
                nc.vector.tensor_tensor(mu[:, 2:3], mu[:, 0:1], mu[:, 0:1], op=OP.mult)
                nc.vector.tensor_tensor(mu[:, 1:2], mu[:, 1:2], mu[:, 2:3], op=OP.subtract)  # var
                nc.vector.tensor_scalar(mu[:, 1:2], mu[:, 1:2], float(cfg["BN_EPS"]), None,
                                        op0=OP.add)
                nc.scalar.activation(mu[:, 1:2], mu[:, 1:2], AF.Sqrt, bias=0.0, scale=1.0)
                nc.vector.reciprocal(mu[:, 1:2], mu[:, 1:2])  # rstd
                nc.vector.tensor_tensor(mu[:, 2:3], vcol(phase, l, 2), mu[:, 1:2], op=OP.mult)  # s
                nc.vector.tensor_tensor(mu[:, 3:4], mu[:, 0:1], mu[:, 2:3], op=OP.mult)
                nc.vector.tensor_tensor(mu[:, 3:4], vcol(phase, l, 3), mu[:, 3:4], op=OP.subtract)  # t

                # normalize + residual + transpose-store
                stgt = stgp.tile([128, 16, 128], BF16, tag="stg")
                stg_fill = 0
                stg_t0 = 0
                for g in range((Sp + 511) // 512):
                    cols = min(512, Sp - g * 512)
                    hg = zzp.tile([128, 512], BF16, tag="hg2")
                    nc.sync.dma_start(hg[:, :cols], hown[g * 512:g * 512 + cols, :],
                                      transpose=True)
                    hn = zzp.tile([128, 512], BF16, tag="hn")
                    nc.vector.tensor_scalar(hn[:, :cols], z2[:, g * 512:g * 512 + cols],
                                            mu[:, 2:3], mu[:, 3:4], op0=OP.mult, op1=OP.add)
                    nc.vector.tensor_tensor(hn[:, :cols], hn[:, :cols], hg[:, :cols], op=OP.add)
                    for q in range(cols // 128):
                        t = g * 4 + q
                        pt = psT.tile([128, 128], BF16, tag="tr")
                        nc.tensor.transpose(pt[:], hn[:, q * 128:(q + 1) * 128], idn[:])
                        nc.vector.tensor_copy(stgt[:, stg_fill, :], pt[:])
                        stg_fill += 1
                        if stg_fill == 16 or t == n_tiles - 1:
                            nc.sync.dma_start(
                                hown.ap().rearrange("(c p) h -> p c h", p=128)[:, stg_t0:stg_t0 + stg_fill, :],
                                stgt[:, :stg_fill, :])
                            stg_t0 += stg_fill
                            stg_fill = 0
                            if t != n_tiles - 1:
                                stgt = stgp.tile([128, 16, 128], BF16, tag="stg")
                return

            # ---- sub phase ----
            for l in range(L):
                gin_layer(0, l, l == L - 1)

            # ---- phase boundary: weighted mean of 2 roots -> hown_glob ----
            r0_sb = idxp.tile([128, S_glob // 16], I16, tag="r0")
            r4_sb = idxp.tile([128, S_glob // 16], I16, tag="r4")
            nc.sync.dma_start(r0_sb[:], t_r0[:])
            nc.sync.dma_start(r4_sb[:], t_r4[:])
            r0b = segp.tile([128, S_glob // 128, H], BF16, tag="seg")
            r4b = segp.tile([128, S_glob // 128, H], BF16, tag="seg")
            nc.gpsimd.dma_gather(r0b[:], hown_sub[:], r0_sb[:], S_glob, S_glob, H,
                                 single_packet=False, queue_num=nextq())
            nc.gpsimd.dma_gather(r4b[:], hown_sub[:], r4_sb[:], S_glob, S_glob, H,
                                 single_packet=False, queue_num=nextq())
            inv_temp = 1.0 / float(cfg["TEMP"])
            for t in range(S_glob // 128):
                lpt = smp.tile([128, 2], F32, tag="lpt")
                nc.sync.dma_start(lpt[:], t_lp[t * 128:(t + 1) * 128, :])
                d = smp.tile([128, 2], F32, tag="d")
                nc.vector.tensor_tensor(d[:, 0:1], lpt[:, 0:1], lpt[:, 1:2], op=OP.subtract)
                nc.scalar.activation(d[:, 0:1], d[:, 0:1], AF.Sigmoid, bias=0.0,
                                     scale=inv_temp)       # w0
                nc.vector.tensor_scalar(d[:, 1:2], d[:, 0:1], -1.0, 1.0,
                                        op0=OP.mult, op1=OP.add)  # w1 = 1-w0
                hb = segp.tile([128, H], BF16, tag="hb")
                nc.vector.tensor_scalar(hb[:], r0b[:, t, :], d[:, 0:1], None, op0=OP.mult)
                nc.vector.scalar_tensor_tensor(hb[:], r4b[:, t, :], d[:, 1:2], hb[:],
                                               op0=OP.mult, op1=OP.add)
                nc.sync.dma_start(hown_glob[t * 128:(t + 1) * 128, :], hb[:])

            # ---- glob phase ----
            for l in range(L):
                gin_layer(1, l, l == L - 1)

            # ---- readout ----
            Srd_sb = constp.tile([128, (S_glob // 128) * G], BF16, tag="srd")
            nc.sync.dma_start(Srd_sb[:], t_Srd[:])
            prd = psM.tile([128, G], F32, tag="m1")
            for t in range(S_glob // 128):
                hrow = segp.tile([128, H], BF16, tag="hrow")
                nc.sync.dma_start(hrow[:], hown_glob[t * 128:(t + 1) * 128, :])
                nc.tensor.matmul(prd[:], hrow[:], Srd_sb[:, t * G:(t + 1) * G],
                                 start=(t == 0), stop=(t == S_glob // 128 - 1))
            rd_sb = constp.tile([128, G], F32, tag="rdsb")
            nc.vector.tensor_copy(rd_sb[:], prd[:])
            nc.sync.dma_start(rd_in[:], rd_sb[:])
            nc.gpsimd.collective_compute(
                "AllReduce", OP.add, RG, [rd_in.ap().opt()], [rd_out.ap().opt()])
            rd2 = constp.tile([128, G], F32, tag="rd2")
            nc.sync.dma_start(rd2[:], rd_out[:])
            idf = constp.tile([128, 128], F32, tag="idf")
            nc.sync.dma_start(idf[:], t_idn_f[:])
            for g in range((G + 127) // 128):
                cols = min(128, G - g * 128)
                pt = psT.tile([128, 128], F32, tag="trf")
                nc.tensor.transpose(pt[:cols, :], rd2[:, g * 128:g * 128 + cols], idf[:])
                ot = constp.tile([128, 128], F32, tag="ot")
                nc.vector.tensor_copy(ot[:cols, :], pt[:cols, :])
                nc.sync.dma_start(t_out[g * 128:g * 128 + cols, :], ot[:cols, :])

    nc.compile()
    return nc


def build_inmaps(plan, weights):
    """weights: dict with all the learned params (numpy f32)."""
    cfg = plan["cfg"]
    W, H, L = cfg["W"], cfg["H"], cfg["L"]
    n_glob, S_glob = cfg["n_glob"], cfg["S_glob"]
    lp = np.asarray(weights["log_probs"], np.float32)
    maps = []
    vecs = np.zeros((128, 10 * L), np.float32)
    for ph, pre in ((0, "sub"), (1, "glob")):
        for l in range(L):
            base = ph * 5 * L + l * 5
            vecs[:, base + 0] = np.asarray(weights[f"{pre}_b1"][l], np.float32)
            vecs[:, base + 1] = np.asarray(weights[f"{pre}_b2"][l], np.float32)
            vecs[:, base + 2] = np.asarray(weights[f"{pre}_gamma"][l], np.float32)
            vecs[:, base + 3] = np.asarray(weights[f"{pre}_beta"][l], np.float32)
            vecs[:, base + 4] = 1.0 + np.float32(weights[f"{pre}_eps"][l])
    idn = np.eye(128)
    common = {
        "Ssub": None, "Sglob": None,
        "atom": np.asarray(weights["atom_table"], np.float32).astype(BF),
        "W1s": np.asarray(weights["sub_W1"], np.float32).astype(BF),
        "W2s": np.asarray(weights["sub_W2"], np.float32).astype(BF),
        "W1g": np.asarray(weights["glob_W1"], np.float32).astype(BF),
        "W2g": np.asarray(weights["glob_W2"], np.float32).astype(BF),
        "vecs": vecs,
        "idnbf": idn.astype(BF),
        "idnf": idn.astype(np.float32),
    }
    for c in range(W):
        lpc = np.zeros((S_glob, 2), np.float32)
        lpc[:n_glob] = lp[plan["lp_sel"][c]]
        m = dict(common)
        m.update({
            "g1idx": plan["sub_g1_idx"][c],
            "g2idx": plan["sub_g2_idx"][c],
            "ggidx": plan["glob_g2_idx"][c],
            "aid": plan["aid"][c],
            "r0idx": plan["r0"][c],
            "r4idx": plan["r4"][c],
            "Ssub": plan["sub_S"][c],
            "Sglob": plan["glob_S"][c],
            "Srd": plan["Sg"][c],
            "lp": lpc,
        })
        maps.append(m)
    return maps


def kernel(**inputs):
    import numpy as np
    cfg = dict(DEF_CFG)
    inp = {k: np.asarray(v) for k, v in inputs.items()}
    plan = build_plan(cfg, inp["x"], inp["edge_index"], inp["sub_node_map"],
                      inp["sub_edge_index"], inp["root_idx"], inp["target_batch"],
                      inp["batch"])
    nc = build_graph(plan)
    maps = build_inmaps(plan, inp)
    from concourse import bass_utils
    res = bass_utils.run_bass_kernel_spmd(nc, maps, core_ids=list(range(cfg["W"])),
                                          trace=False)
    return np.asarray(res.results[0]["out"], np.float32)



# revision 13
# speedup vs baseline: 1.1823x; 1.1823x over previous
"""Trainium2 Bass kernel: distributed GIN graph encoder on 8 NeuronCores.

Self-contained: host-side graph partitioning + index/one-hot table construction,
Bass/Tile graph build, SPMD execution via run_bass_kernel_spmd, result gather.

v1: single-stage edge gathers (no stage-1 bounce), 256-col aggregation bands.
"""

import numpy as np
import ml_dtypes

BF = ml_dtypes.bfloat16

DEF_CFG = dict(
    W=8, H=128, L=4,
    n_sub=30000, S_sub=30080,     # per-core real/padded sub rows
    n_glob=3750, S_glob=3840,
    VA=128, G=300, TEMP=0.5, BN_EPS=1e-5,
    GB=256,                       # dst-group band width (cols) for sub agg
    SPAN=8,                       # 256-groups per gather call span
    GRP=16,                       # glob: chunks per gather call
)


def _pack16(arr):
    """idx array (n,) int -> [128, n/16] int16 tile content (pos i -> [i%16, i//16])."""
    a = np.asarray(arr, np.int16)
    assert len(a) % 16 == 0
    t = a.reshape(-1, 16).T
    return np.tile(t, (8, 1))


def _pad128(n):
    return (n + 127) // 128 * 128


def build_plan(cfg, x, edge_index, sub_node_map, sub_edge_index, root_idx,
               target_batch, batch):
    """All index tables / budgets, shared-static structure + per-core data."""
    W, H = cfg["W"], cfg["H"]
    n_sub, S_sub = cfg["n_sub"], cfg["S_sub"]
    n_glob, S_glob = cfg["n_glob"], cfg["S_glob"]
    GB, SPAN = cfg["GB"], cfg["SPAN"]
    N = n_glob * W
    plan = {"cfg": cfg}

    # ---------- sub phase edge plan (single stage, direct from replica) ----
    # groups of GB dst cols; calls per (span of SPAN groups, src bucket)
    src, dst = np.asarray(sub_edge_index[0]), np.asarray(sub_edge_index[1])
    owner = dst // n_sub
    dst_local = dst % n_sub
    src_row = (src // n_sub) * S_sub + (src % n_sub)   # row in padded replica
    n_groups = (S_sub + GB - 1) // GB                  # 118 for GB=256
    n_spans = (n_groups + SPAN - 1) // SPAN

    # per (core, group, bucket) edge lists
    per_cgb = {}
    cnt = np.zeros((W, n_groups, W), np.int64)
    for c in range(W):
        m = owner == c
        sc, dl = src_row[m], dst_local[m]
        g = dl // GB
        b = sc // S_sub
        order = np.lexsort((sc, b, g))
        sc, dl, g, b = sc[order], dl[order], g[order], b[order]
        # boundaries per (g,b)
        key = g * W + b
        uniq, start, counts = np.unique(key, return_index=True, return_counts=True)
        for u, st0, k in zip(uniq, start, counts):
            per_cgb[(c, int(u) // W, int(u) % W)] = (sc[st0:st0 + k], dl[st0:st0 + k])
            cnt[c, int(u) // W, int(u) % W] = k
    budget = cnt.max(axis=0)                            # [n_groups, W]
    chunks = np.maximum((budget + 127) // 128, 1)       # [n_groups, W]

    # gather slot layout: span-major, bucket-major within span, group within.
    # call (span, b) covers chunks of groups [span*SPAN, ...) for bucket b
    gchunk_off = np.zeros((n_groups, W), np.int64)      # gather chunk idx of (g,b)
    call_list = []                                      # per span: list of (b, chunk0, nch)
    tot_chunks = 0
    for s in range(n_spans):
        g0, g1 = s * SPAN, min((s + 1) * SPAN, n_groups)
        calls = []
        for b in range(W):
            c0 = tot_chunks
            for g in range(g0, g1):
                gchunk_off[g, b] = tot_chunks
                tot_chunks += int(chunks[g, b])
            calls.append((b, c0, tot_chunks - c0))
        call_list.append(calls)
    # S layout: group-major (all of group g's chunks contiguous, bucket order)
    schunk_base = np.zeros(n_groups + 1, np.int64)
    schunk_base[1:] = np.cumsum(chunks.sum(axis=1))
    assert int(schunk_base[-1]) == tot_chunks
    plan["sub_calls"] = call_list
    plan["sub_tot_chunks"] = tot_chunks
    plan["sub_chunks"] = chunks
    plan["sub_gchunk_off"] = gchunk_off
    plan["sub_schunk_base"] = schunk_base
    plan["sub_n_groups"] = n_groups
    plan["sub_n_spans"] = n_spans
    plan["sub_max_call_nch"] = max(nch for calls in call_list for (_, _, nch) in calls)
    plan["sub_max_grp_nch"] = int(chunks.sum(axis=1).max())
    plan["sub_max_span_nch"] = max(sum(nch for (_, _, nch) in calls)
                                   for calls in call_list)

    # per-core idx + S content
    TOT = tot_chunks * 128
    sub_idx_cores, sub_S_cores = [], []
    for c in range(W):
        idx = np.zeros(TOT, np.int64)
        S = np.zeros((TOT, GB), BF)
        for g in range(n_groups):
            scb = int(schunk_base[g])
            for b in range(W):
                sc, dl = per_cgb.get((c, g, b), (np.zeros(0, np.int64),) * 2)
                k = len(sc)
                idx[gchunk_off[g, b] * 128:gchunk_off[g, b] * 128 + k] = sc % S_sub
                S[scb * 128 + np.arange(k), dl % GB] = BF(1.0)
                scb += int(chunks[g, b])
        sub_idx_cores.append(_pack16(idx))
        sub_S_cores.append(
            S.reshape(tot_chunks, 128, GB).transpose(1, 0, 2).reshape(128, tot_chunks * GB))
    plan["sub_idx"] = sub_idx_cores
    plan["sub_S"] = sub_S_cores

    # ---------- glob phase edge plan (single stage, as before) ----------
    gsrc, gdst = np.asarray(edge_index[0]), np.asarray(edge_index[1])
    gowner = gdst // n_glob
    gdst_local = gdst % n_glob
    gsrc_row = (gsrc // n_glob) * S_glob + (gsrc % n_glob)
    n_tiles_glob = S_glob // 128
    per_core_g = []
    CtG = np.zeros(n_tiles_glob, np.int64)
    for c in range(W):
        m = gowner == c
        sc, dl = gsrc_row[m], gdst_local[m]
        order = np.lexsort((sc, dl // 128))
        sc, dl = sc[order], dl[order]
        per_core_g.append((sc, dl))
        cnt2 = np.bincount(dl // 128, minlength=n_tiles_glob)
        CtG = np.maximum(CtG, (cnt2 + 127) // 128)
    CtG = np.maximum(CtG, 1)
    plan["glob_Ct"] = CtG
    TOTCHG = int(CtG.sum())
    plan["glob_TOTCH"] = TOTCHG
    tile_slot_off_g = np.zeros(n_tiles_glob + 1, np.int64)
    tile_slot_off_g[1:] = np.cumsum(CtG * 128)
    gg_idx_cores, Sg_cores = [], []
    for c in range(W):
        sc, dl = per_core_g[c]
        g2 = np.zeros(TOTCHG * 128, np.int64)
        S = np.zeros((TOTCHG * 128, 128), BF)
        tl = dl // 128
        for t in range(n_tiles_glob):
            mt = tl == t
            k = int(mt.sum())
            slots = tile_slot_off_g[t] + np.arange(k)
            g2[slots] = sc[mt]
            S[slots, dl[mt] % 128] = BF(1.0)
        gg_idx_cores.append(_pack16(g2))
        Sg_cores.append(S)
    plan["glob_g2_idx"] = gg_idx_cores
    plan["glob_S"] = [S.reshape(TOTCHG, 128, 128).transpose(1, 0, 2).reshape(128, TOTCHG * 128)
                      for S in Sg_cores]

    # ---------- atom encode ----------
    aid = np.asarray(x)[np.asarray(sub_node_map)]      # [NS] atom id per sub node
    plan["aid"] = []
    for c in range(W):
        a = np.zeros(S_sub, np.int64)
        a[:n_sub] = aid[c * n_sub:(c + 1) * n_sub]
        plan["aid"].append(_pack16(a))

    # ---------- phase boundary (roots) ----------
    tb_arr = np.asarray(target_batch)
    ri = np.asarray(root_idx)
    order = np.argsort(tb_arr, kind="stable")
    assert (np.bincount(tb_arr, minlength=N) == 2).all(), "need exactly 2 roots/node"
    r_sorted = ri[order].reshape(N, 2)      # root rows (in hsub global) per node
    lp_order = order.reshape(N, 2)          # positions into log_probs
    plan["r0"], plan["r4"], plan["lp_sel"] = [], [], []
    for c in range(W):
        r = r_sorted[c * n_glob:(c + 1) * n_glob]
        lo = c * n_sub
        assert ((r >= lo) & (r < lo + n_sub)).all(), "roots must be core-local"
        r0 = np.zeros(S_glob, np.int64)
        r4 = np.zeros(S_glob, np.int64)
        r0[:n_glob] = r[:, 0] - lo
        r4[:n_glob] = r[:, 1] - lo
        plan["r0"].append(_pack16(r0))
        plan["r4"].append(_pack16(r4))
        plan["lp_sel"].append(lp_order[c * n_glob:(c + 1) * n_glob])  # [n_glob,2]

    # ---------- readout ----------
    b_arr = np.asarray(batch)
    plan["Sg"] = []
    for c in range(W):
        Srd = np.zeros((S_glob, cfg["G"]), BF)
        ids = b_arr[c * n_glob:(c + 1) * n_glob]
        Srd[np.arange(n_glob), ids] = BF(1.0)
        nt = S_glob // 128
        plan["Sg"].append(Srd.reshape(nt, 128, cfg["G"]).transpose(1, 0, 2).reshape(128, nt * cfg["G"]))
    return plan


def _install_queue_aware_lanes():
    """Make Tile's DMASW lane assignment queue-aware: lane = queue*2 + rr.
    Needed because we spread dma_gather over 4 SWDGE queues; the stock
    assigner round-robins 8 lanes queue-blind, which trips the per-queue
    sem lock in ucode/sim."""
    import concourse.tile_sem_assignment as tsa
    if getattr(tsa, "_qaware_installed", False):
        return
    orig = tsa.TileClockTick._assign_tick
    import concourse.mybir as mb

    def patched(self, inst):
        qn = getattr(inst, "queue_num", None)
        if (qn is not None and inst.engine == mb.EngineType.Pool
                and isinstance(inst, tsa.DMAInst)
                and self.swdge_sem_count == 8):
            rr_map = getattr(self, "_q_rr", None)
            if rr_map is None:
                rr_map = self._q_rr = {}
            sub = rr_map.get(qn, 0)
            rr_map[qn] = (sub + 1) % 2
            lane = qn * 2 + sub
            save = self.next_sw_dma_idx
            self.next_sw_dma_idx = lane
            try:
                return orig(self, inst)
            finally:
                self.next_sw_dma_idx = save
        return orig(self, inst)

    tsa.TileClockTick._assign_tick = patched
    tsa._qaware_installed = True


def build_graph(plan):
    from concourse import bass, mybir, bacc
    import concourse.tile as tile

    cfg = plan["cfg"]
    W, H, L = cfg["W"], cfg["H"], cfg["L"]
    n_sub, S_sub = cfg["n_sub"], cfg["S_sub"]
    n_glob, S_glob = cfg["n_glob"], cfg["S_glob"]
    G = cfg["G"]
    GB = cfg["GB"]
    BF16 = mybir.dt.bfloat16
    F32 = mybir.dt.float32
    I16 = mybir.dt.int16
    AF = mybir.ActivationFunctionType
    OP = mybir.AluOpType
    GRP = cfg["GRP"]

    _install_queue_aware_lanes()
    nc = bacc.Bacc("TRN2", target_bir_lowering=False, debug=False, num_devices=W,
                   num_swdge_queues=4)

    # ---- inputs ----
    def inp(name, shape, dt):
        return nc.dram_tensor(name, shape, dt, kind="ExternalInput")

    TOTCH = plan["sub_tot_chunks"]
    TOTCHG = plan["glob_TOTCH"]
    t_gs = inp("gsidx", [128, TOTCH * 8], I16)          # sub edge idx (packed)
    t_gg = inp("ggidx", [128, TOTCHG * 8], I16)
    t_aid = inp("aid", [128, S_sub // 16], I16)
    t_r0 = inp("r0idx", [128, S_glob // 16], I16)
    t_r4 = inp("r4idx", [128, S_glob // 16], I16)
    t_Ssub = inp("Ssub", [128, TOTCH * GB], BF16)
    t_Sglob = inp("Sglob", [128, TOTCHG * 128], BF16)
    t_Srd = inp("Srd", [128, (S_glob // 128) * G], BF16)
    t_idn_bf = inp("idnbf", [128, 128], BF16)
    t_idn_f = inp("idnf", [128, 128], F32)
    t_atom = inp("atom", [cfg["VA"], H], BF16)
    t_W1s = inp("W1s", [L, H, H], BF16)
    t_W2s = inp("W2s", [L, H, H], BF16)
    t_W1g = inp("W1g", [L, H, H], BF16)
    t_W2g = inp("W2g", [L, H, H], BF16)
    t_vecs = inp("vecs", [128, 10 * L], F32)   # b1,b2,gam,bet,eps per layer x(sub,glob)
    t_lp = inp("lp", [S_glob, 2], F32)
    t_out = nc.dram_tensor("out", [G, H], F32, kind="ExternalOutput")

    # ---- internal DRAM ----
    rep_sub = nc.dram_tensor("rep_sub", [W * S_sub, H], BF16, addr_space="Shared")
    rep_glob = nc.dram_tensor("rep_glob", [W * S_glob, H], BF16, addr_space="Shared")
    hown_sub = nc.dram_tensor("hown_sub", [S_sub, H], BF16)
    hown_glob = nc.dram_tensor("hown_glob", [S_glob, H], BF16)
    ar_in = nc.dram_tensor("ar_in", [128, 2], F32)
    ar_out = nc.dram_tensor("ar_out", [128, 2], F32, addr_space="Shared")
    rd_in = nc.dram_tensor("rd_in", [128, G], F32)
    rd_out = nc.dram_tensor("rd_out", [128, G], F32, addr_space="Shared")

    RG = [list(range(W))]
    _qrr = [0]

    def nextq():
        # must mirror tile_sem_assignment's DMASW lane rotation (8 lanes,
        # advanced per Pool-engine DMA inst in trace order): lane i -> queue i//2
        q = (_qrr[0] % 8) // 2
        _qrr[0] += 1
        return q

    sub_calls = plan["sub_calls"]
    sub_chunks = plan["sub_chunks"]
    sub_gchunk_off = plan["sub_gchunk_off"]
    schunk_base = plan["sub_schunk_base"]
    n_groups = plan["sub_n_groups"]
    n_spans = plan["sub_n_spans"]
    SPAN = cfg["SPAN"]
    CtG = plan["glob_Ct"]

    with tile.TileContext(nc) as tc:
        with (
            tc.tile_pool(name="const", bufs=1) as constp,
            tc.tile_pool(name="idx", bufs=1) as idxp,
            tc.tile_pool(name="seg", bufs=2) as segp,
            tc.tile_pool(name="xs", bufs=3) as xsp,
            tc.tile_pool(name="zz", bufs=3) as zzp,
            tc.tile_pool(name="res", bufs=1) as resp,
            tc.tile_pool(name="small", bufs=2) as smp,
            tc.tile_pool(name="stg", bufs=2) as stgp,
            tc.tile_pool(name="psA", bufs=8, space="PSUM") as psA,
            tc.tile_pool(name="psM", bufs=1, space="PSUM") as psM,
            tc.tile_pool(name="psT", bufs=2, space="PSUM") as psT,
        ):
            # ---- constants resident ----
            vecs = constp.tile([128, 10 * L], F32)
            nc.sync.dma_start(vecs[:], t_vecs[:])
            W1s = constp.tile([128, L * H], BF16)
            W2s = constp.tile([128, L * H], BF16)
            W1g = constp.tile([128, L * H], BF16)
            W2g = constp.tile([128, L * H], BF16)
            for l in range(L):
                nc.sync.dma_start(W1s[:, l * H:(l + 1) * H], t_W1s[l])
                nc.sync.dma_start(W2s[:, l * H:(l + 1) * H], t_W2s[l])
                nc.sync.dma_start(W1g[:, l * H:(l + 1) * H], t_W1g[l])
                nc.sync.dma_start(W2g[:, l * H:(l + 1) * H], t_W2g[l])
            idn = constp.tile([128, 128], BF16, tag="idn")
            nc.sync.dma_start(idn[:], t_idn_bf[:])

            # vecs columns: per l: [b1, b2, gamma, beta, eps] sub then glob
            def vcol(phase, l, j):
                return vecs[:, (phase * 5 * L + l * 5 + j):(phase * 5 * L + l * 5 + j) + 1]

            # ---- atom encode -> hown_sub ----
            aid_sb = idxp.tile([128, S_sub // 16], I16, tag="aid")
            nc.sync.dma_start(aid_sb[:], t_aid[:])
            CALL = 4096
            off = 0
            while off < S_sub:
                n = min(CALL, S_sub - off)
                at = segp.tile([128, CALL // 128, H], BF16, tag="seg")
                nc.gpsimd.dma_gather(at[:, :n // 128, :], t_atom[:],
                                     aid_sb[:, off // 16:(off + n) // 16], n, n, H,
                                     single_packet=False, queue_num=nextq())
                nc.sync.dma_start(
                    hown_sub.ap().rearrange("(c p) h -> p c h", p=128)[:, off // 128:(off + n) // 128, :],
                    at[:, :n // 128, :])
                off += n

            def gin_layer(phase, l):
                """One GIN layer. phase 0=sub, 1=glob."""
                if phase == 0:
                    Sp, n_real, rep, hown = S_sub, n_sub, rep_sub, hown_sub
                    Wt1, Wt2 = W1s, W2s
                else:
                    Sp, n_real, rep, hown = S_glob, n_glob, rep_glob, hown_glob
                    Wt1, Wt2 = W1g, W2g
                n_tiles = Sp // 128

                # AllGather h -> replica
                nc.gpsimd.collective_compute(
                    "AllGather", OP.bypass, RG, [hown.ap().opt()], [rep.ap().opt()])

                z2 = resp.tile([128, S_sub], BF16, tag="z2")     # max-size shared
                stats = smp.tile([128, 64 * 6], F32, tag="stats")

                n_groups_512 = (Sp + 511) // 512
                zgrp_tiles = {}

                def run_mlp(g):
                    zg = zgrp_tiles.pop(g)
                    cols = min(512, Sp - g * 512)
                    pm = psM.tile([128, 512], F32, tag="m1")
                    nc.tensor.matmul(pm[:, :cols], Wt1[:, l * H:(l + 1) * H], zg[:, :cols],
                                     start=True, stop=True)
                    z1 = zzp.tile([128, 512], BF16, tag="z1")
                    nc.scalar.activation(z1[:, :cols], pm[:, :cols], AF.Relu,
                                         bias=vcol(phase, l, 0), scale=1.0)
                    pm2 = psM.tile([128, 512], F32, tag="m2")
                    nc.tensor.matmul(pm2[:, :cols], Wt2[:, l * H:(l + 1) * H], z1[:, :cols],
                                     start=True, stop=True)
                    nc.vector.tensor_scalar(z2[:, g * 512:g * 512 + cols], pm2[:, :cols],
                                            vcol(phase, l, 1), None, op0=OP.add)
                    realc = min(512, max(0, n_real - g * 512))
                    if realc > 0:
                        nc.vector.bn_stats(stats[:, g * 6:(g + 1) * 6],
                                           z2[:, g * 512:g * 512 + realc])

                def get_zg(g):
                    """Materialize the 512-supergroup tile + its h^T load."""
                    if g not in zgrp_tiles:
                        zgrp_tiles[g] = zzp.tile([128, 512], BF16, tag="zg", name=f"zg{g%4}", bufs=1)
                        hg = zzp.tile([128, 512], BF16, tag="hg")
                        cols = min(512, Sp - g * 512)
                        nc.sync.dma_start(hg[:, :cols], hown[g * 512:g * 512 + cols, :],
                                          transpose=True)
                        zgrp_tiles[(g, "h")] = hg
                    return zgrp_tiles[g], zgrp_tiles[(g, "h")]

                done_cols = [0]

                def epilogue(pt, col0, cols):
                    """(1+eps)*h^T + agg^T for columns [col0, col0+cols)."""
                    g = col0 // 512
                    zg, hg = get_zg(g)
                    cc = col0 - g * 512
                    nc.vector.scalar_tensor_tensor(
                        zg[:, cc:cc + cols], hg[:, cc:cc + cols],
                        vcol(phase, l, 4), pt[:, :cols],
                        op0=OP.mult, op1=OP.add)
                    done_cols[0] += cols
                    if (col0 + cols) % 512 == 0 or col0 + cols == Sp:
                        zgrp_tiles.pop((g, "h"))
                        run_mlp(g)

                if phase == 0:
                    # ---- sub aggregation: span calls, 256-col band matmuls ----
                    gs_sb = idxp.tile([128, TOTCH * 8], I16, tag="gidx")
                    nc.sync.dma_start(gs_sb[:], t_gs[:])
                    MAXSP = plan["sub_max_span_nch"]
                    MAXG = plan["sub_max_grp_nch"]
                    for s in range(n_spans):
                        g0, g1 = s * SPAN, min((s + 1) * SPAN, n_groups)
                        span_c0 = sub_calls[s][0][1]
                        xt = xsp.tile([128, MAXSP, H], BF16, tag="xsub", bufs=2)
                        for (b, c0, nch) in sub_calls[s]:
                            nc.gpsimd.dma_gather(
                                xt[:, c0 - span_c0:c0 - span_c0 + nch, :],
                                rep[b * S_sub:(b + 1) * S_sub, :],
                                gs_sb[:, c0 * 8:(c0 + nch) * 8],
                                nch * 128, nch * 128, H,
                                single_packet=False, queue_num=nextq())
                        # matmuls per group; S streamed group-major
                        for g in range(g0, g1):
                            sc0 = int(schunk_base[g])
                            snch = int(schunk_base[g + 1]) - sc0
                            st = xsp.tile([128, MAXG, GB], BF16, tag="ssub", bufs=2)
                            nc.sync.dma_start(
                                st[:, :snch, :],
                                t_Ssub.ap().rearrange("p (c g) -> p c g", g=GB)[:, sc0:sc0 + snch, :])
                            pt = psA.tile([128, GB], F32, tag="agg", name=f"agg{g%4}",
                                          bufs=1)
                            first = True
                            si = 0
                            for b in range(W):
                                for j in range(int(sub_chunks[g, b])):
                                    gc = int(sub_gchunk_off[g, b]) + j
                                    nc.tensor.matmul(
                                        pt[:], xt[:, gc - span_c0, :], st[:, si, :],
                                        start=first,
                                        stop=(b == W - 1 and j == int(sub_chunks[g, b]) - 1))
                                    first = False
                                    si += 1
                            cols = min(GB, Sp - g * GB)
                            epilogue(pt, g * GB, cols)
                else:
                    # ---- glob aggregation: per-tile chunks as before ----
                    gi_sb = idxp.tile([128, TOTCH * 8], I16, tag="gidx")
                    nc.sync.dma_start(gi_sb[:, :TOTCHG * 8], t_gg[:])
                    chunks_l = []   # (tile, first, last)
                    for t in range(n_tiles):
                        for j in range(int(CtG[t])):
                            chunks_l.append((t, j == 0, j == int(CtG[t]) - 1))
                    psum_of = {}
                    groups = [chunks_l[i:i + GRP] for i in range(0, len(chunks_l), GRP)]
                    ch_base = 0
                    for grp in groups:
                        nch = len(grp)
                        xt = xsp.tile([128, GRP, H], BF16, tag="x", bufs=2)
                        n = nch * 128
                        nc.gpsimd.dma_gather(
                            xt[:, :nch, :], rep[:, :],
                            gi_sb[:, ch_base * 8:(ch_base + nch) * 8],
                            n, n, H, single_packet=False, queue_num=nextq())
                        st = xsp.tile([128, GRP * 128], BF16, tag="s", bufs=2)
                        nc.sync.dma_start(st[:, :n], t_Sglob[:, ch_base * 128:ch_base * 128 + n])
                        for j, (t, first, lastc) in enumerate(grp):
                            if first:
                                psum_of[t] = psA.tile([128, 128], F32, tag="aggG",
                                                      name=f"aggG{t%8}", bufs=1)
                            nc.tensor.matmul(psum_of[t][:], xt[:, j, :],
                                             st[:, j * 128:(j + 1) * 128],
                                             start=first, stop=lastc)
                            if lastc:
                                pt = psum_of.pop(t)
                                epilogue(pt, t * 128, 128)
                        ch_base += nch

                # BN: aggregate + allreduce
                ngr = (n_real + 511) // 512
                mv = smp.tile([128, 2], F32, tag="mv")
                nc.vector.bn_aggr(mv[:], stats[:, :ngr * 6])
                sin = smp.tile([128, 2], F32, tag="sin")
                # sin = [mean, var + mean^2]
                nc.vector.tensor_tensor(sin[:, 1:2], mv[:, 0:1], mv[:, 0:1], op=OP.mult)
                nc.vector.tensor_tensor(sin[:, 1:2], sin[:, 1:2], mv[:, 1:2], op=OP.add)
                nc.vector.tensor_copy(sin[:, 0:1], mv[:, 0:1])
                nc.sync.dma_start(ar_in[:], sin[:])
                nc.gpsimd.collective_compute(
                    "AllReduce", OP.add, RG, [ar_in.ap().opt()], [ar_out.ap().opt()])
                sg = smp.tile([128, 2], F32, tag="sg")
                nc.sync.dma_start(sg[:], ar_out[:])
                mu = smp.tile([128, 4], F32, tag="mu")
                nc.scalar.mul(mu[:, 0:1], sg[:, 0:1], 1.0 / W)          # mu
                nc.scalar.mul(mu[:, 1:2], sg[:, 1:2], 1.0 / W)          # E[z2]
                nc.vector.tensor_tensor(mu[:, 2:3], mu[:, 0:1], mu[:, 0:1], op=OP.mult)
                nc.vector.tensor_tensor(mu[:, 1:2], mu[:, 1:2], mu[:, 2:3], op=OP.subtract)  # var
                nc.vector.tensor_scalar(mu[:, 1:2], mu[:, 1:2], float(cfg["BN_EPS"]), None,
                                        op0=OP.add)
                nc.scalar.activation(mu[:, 1:2], mu[:, 1:2], AF.Sqrt, bias=0.0, scale=1.0)
                nc.vector.reciprocal(mu[:, 1:2], mu[:, 1:2])  # rstd
                nc.vector.tensor_tensor(mu[:, 2:3], vcol(phase, l, 2), mu[:, 1:2], op=OP.mult)  # s
                nc.vector.tensor_tensor(mu[:, 3:4], mu[:, 0:1], mu[:, 2:3], op=OP.mult)
                nc.vector.tensor_tensor(mu[:, 3:4], vcol(phase, l, 3), mu[:, 3:4], op=OP.subtract)  # t

                # normalize + residual + transpose-store
                stgt = stgp.tile([128, 16, 128], BF16, tag="stg")
                stg_fill = 0
                stg_t0 = 0
                for g in range((Sp + 511) // 512):
                    cols = min(512, Sp - g * 512)
                    hg = zzp.tile([128, 512], BF16, tag="hg2")
                    nc.sync.dma_start(hg[:, :cols], hown[g * 512:g * 512 + cols, :],
                                      transpose=True)
                    hn = zzp.tile([128, 512], BF16, tag="hn")
                    nc.vector.tensor_scalar(hn[:, :cols], z2[:, g * 512:g * 512 + cols],
                                            mu[:, 2:3], mu[:, 3:4], op0=OP.mult, op1=OP.add)
                    nc.vector.tensor_tensor(hn[:, :cols], hn[:, :cols], hg[:, :cols], op=OP.add)
                    for q in range(cols // 128):
                        t = g * 4 + q
                        pt = psT.tile([128, 128], BF16, tag="tr")
                        nc.tensor.transpose(pt[:], hn[:, q * 128:(q + 1) * 128], idn[:])
                        nc.vector.tensor_copy(stgt[:, stg_fill, :], pt[:])
                        stg_fill += 1
                        if stg_fill == 16 or t == n_tiles - 1:
                            nc.sync.dma_start(
                                hown.ap().rearrange("(c p) h -> p c h", p=128)[:, stg_t0:stg_t0 + stg_fill, :],
                                stgt[:, :stg_fill, :])
                            stg_t0 += stg_fill
                            stg_fill = 0
                            if t != n_tiles - 1:
                                stgt = stgp.tile([128, 16, 128], BF16, tag="stg")
                return

            # ---- sub phase ----
            for l in range(L):
                gin_layer(0, l)

            # ---- phase boundary: weighted mean of 2 roots -> hown_glob ----
            r0_sb = idxp.tile([128, S_glob // 16], I16, tag="r0")
            r4_sb = idxp.tile([128, S_glob // 16], I16, tag="r4")
            nc.sync.dma_start(r0_sb[:], t_r0[:])
            nc.sync.dma_start(r4_sb[:], t_r4[:])
            r0b = segp.tile([128, S_glob // 128, H], BF16, tag="seg")
            r4b = segp.tile([128, S_glob // 128, H], BF16, tag="seg")
            nc.gpsimd.dma_gather(r0b[:], hown_sub[:], r0_sb[:], S_glob, S_glob, H,
                                 single_packet=False, queue_num=nextq())
            nc.gpsimd.dma_gather(r4b[:], hown_sub[:], r4_sb[:], S_glob, S_glob, H,
                                 single_packet=False, queue_num=nextq())
            inv_temp = 1.0 / float(cfg["TEMP"])
            for t in range(S_glob // 128):
                lpt = smp.tile([128, 2], F32, tag="lpt")
                nc.sync.dma_start(lpt[:], t_lp[t * 128:(t + 1) * 128, :])
                d = smp.tile([128, 2], F32, tag="d")
                nc.vector.tensor_tensor(d[:, 0:1], lpt[:, 0:1], lpt[:, 1:2], op=OP.subtract)
                nc.scalar.activation(d[:, 0:1], d[:, 0:1], AF.Sigmoid, bias=0.0,
                                     scale=inv_temp)       # w0
                nc.vector.tensor_scalar(d[:, 1:2], d[:, 0:1], -1.0, 1.0,
                                        op0=OP.mult, op1=OP.add)  # w1 = 1-w0
                hb = segp.tile([128, H], BF16, tag="hb")
                nc.vector.tensor_scalar(hb[:], r0b[:, t, :], d[:, 0:1], None, op0=OP.mult)
                nc.vector.scalar_tensor_tensor(hb[:], r4b[:, t, :], d[:, 1:2], hb[:],
                                               op0=OP.mult, op1=OP.add)
                nc.sync.dma_start(hown_glob[t * 128:(t + 1) * 128, :], hb[:])

            # ---- glob phase ----
            for l in range(L):
                gin_layer(1, l)

            # ---- readout ----
            Srd_sb = constp.tile([128, (S_glob // 128) * G], BF16, tag="srd")
            nc.sync.dma_start(Srd_sb[:], t_Srd[:])
            prd = psM.tile([128, G], F32, tag="m1")
            for t in range(S_glob // 128):
                hrow = segp.tile([128, H], BF16, tag="hrow")
                nc.sync.dma_start(hrow[:], hown_glob[t * 128:(t + 1) * 128, :])
                nc.tensor.matmul(prd[:], hrow[:], Srd_sb[:, t * G:(t + 1) * G],
                                 start=(t == 0), stop=(t == S_glob // 128 - 1))
            rd_sb = constp.tile([128, G], F32, tag="rdsb")
            nc.vector.tensor_copy(rd_sb[:], prd[:])
            nc.sync.dma_start(rd_in[:], rd_sb[:])
            nc.gpsimd.collective_compute(
                "AllReduce", OP.add, RG, [rd_in.ap().opt()], [rd_out.ap().opt()])
            rd2 = constp.tile([128, G], F32, tag="rd2")
            nc.sync.dma_start(rd2[:], rd_out[:])
            idf = constp.tile([128, 128], F32, tag="idf")
            nc.sync.dma_start(idf[:], t_idn_f[:])
            for g in range((G + 127) // 128):
                cols = min(128, G - g * 128)
                pt = psT.tile([128, 128], F32, tag="trf")
                nc.tensor.transpose(pt[:cols, :], rd2[:, g * 128:g * 128 + cols], idf[:])
                ot = constp.tile([128, 128], F32, tag="ot")
                nc.vector.tensor_copy(ot[:cols, :], pt[:cols, :])
                nc.sync.dma_start(t_out[g * 128:g * 128 + cols, :], ot[:cols, :])

    nc.compile()
    return nc


def build_inmaps(plan, weights):
    """weights: dict with all the learned params (numpy f32)."""
    cfg = plan["cfg"]
    W, H, L = cfg["W"], cfg["H"], cfg["L"]
    n_glob, S_glob = cfg["n_glob"], cfg["S_glob"]
    lp = np.asarray(weights["log_probs"], np.float32)
    maps = []
    vecs = np.zeros((128, 10 * L), np.float32)
    for ph, pre in ((0, "sub"), (1, "glob")):
        for l in range(L):
            base = ph * 5 * L + l * 5
            vecs[:, base + 0] = np.asarray(weights[f"{pre}_b1"][l], np.float32)
            vecs[:, base + 1] = np.asarray(weights[f"{pre}_b2"][l], np.float32)
            vecs[:, base + 2] = np.asarray(weights[f"{pre}_gamma"][l], np.float32)
            vecs[:, base + 3] = np.asarray(weights[f"{pre}_beta"][l], np.float32)
            vecs[:, base + 4] = 1.0 + np.float32(weights[f"{pre}_eps"][l])
    idn = np.eye(128)
    common = {
        "atom": np.asarray(weights["atom_table"], np.float32).astype(BF),
        "W1s": np.asarray(weights["sub_W1"], np.float32).astype(BF),
        "W2s": np.asarray(weights["sub_W2"], np.float32).astype(BF),
        "W1g": np.asarray(weights["glob_W1"], np.float32).astype(BF),
        "W2g": np.asarray(weights["glob_W2"], np.float32).astype(BF),
        "vecs": vecs,
        "idnbf": idn.astype(BF),
        "idnf": idn.astype(np.float32),
    }
    for c in range(W):
        lpc = np.zeros((S_glob, 2), np.float32)
        lpc[:n_glob] = lp[plan["lp_sel"][c]]
        m = dict(common)
        m.update({
            "gsidx": plan["sub_idx"][c],
            "ggidx": plan["glob_g2_idx"][c],
            "aid": plan["aid"][c],
            "r0idx": plan["r0"][c],
            "r4idx": plan["r4"][c],
            "Ssub": plan["sub_S"][c],
            "Sglob": plan["glob_S"][c],
            "Srd": plan["Sg"][c],
            "lp": lpc,
        })
        maps.append(m)
    return maps


def kernel(**inputs):
    import numpy as np
    cfg = dict(DEF_CFG)
    inp = {k: np.asarray(v) for k, v in inputs.items()}
    plan = build_plan(cfg, inp["x"], inp["edge_index"], inp["sub_node_map"],
                      inp["sub_edge_index"], inp["root_idx"], inp["target_batch"],
                      inp["batch"])
    nc = build_graph(plan)
    maps = build_inmaps(plan, inp)
    from concourse import bass_utils
    res = bass_utils.run_bass_kernel_spmd(nc, maps, core_ids=list(range(cfg["W"])),
                                          trace=False)
    return np.asarray(res.results[0]["out"], np.float32)
